# revision 1
# baseline (speedup 1.0000x reference)
"""Bahdanau-style attention kernel for Trainium2, data-parallel over batch on
8 NeuronCores.

Reference computation (per batch b):
    enc   = enc_state @ W_enc.T            # [S, H]
    dec   = W_dec @ dec_state              # [H]
    t     = tanh(enc + dec)                # [S, H]
    en    = t @ W_energy.T                 # [S]
    en    = where(arange(S) < L, en, -inf)
    alpha = softmax(en)                    # [S]
    ctx   = alpha @ enc_state              # [2H]

Device mapping (per core, B_LOC=4 batches):
  - Big projection runs on TensorE in float32r (FP22 reads, fp32 PSUM accum,
    full 1 cycle/row speed for N>=256) with W_encT tiles as the stationary
    operand and transposed enc blocks (prepared host-side) as the moving
    operand, producing [h-part, s-free] tiles.
  - ScalarE applies tanh fused with the +dec bias (per-partition bias).
  - Energy reduction over h is an M=1 matmul accumulated over 8 h-tiles.
  - Masking is an additive -1e30 mask (precomputed host-side from
    src_length) fused into the PSUM->SBUF eviction add on VectorE.
  - Softmax runs on a [1, S] row: reduce_max, exp with accumulated sum
    (ACT accum_out), reciprocal, scale.
  - alphas row is transposed to columns with 16 PE transpose ops, and the
    context is an M=1 matmul over natural-layout enc blocks streamed in a
    second pass.
"""

import numpy as np

import concourse.tile as tile
from concourse import bacc, mybir
from concourse.bass_utils import run_bass_kernel_spmd

B, S, H = 32, 2048, 1024
E = 2 * H
NCORES = 8
B_LOC = B // NCORES
P = 128
SBLK = 512
NEG = np.float32(-1e30)

f32 = mybir.dt.float32
f32r = mybir.dt.float32r
AF = mybir.ActivationFunctionType


def build_program(b_loc=B_LOC, s=S, h=H, e=E, sblk=SBLK, n_cores=NCORES, n_iter=1):
    ET = e // P          # e-tiles (contraction tiles of the projection)
    HT = h // P          # h-tiles
    DT = h // P          # d-tiles for the dec projection
    NSB = s // sblk      # s-blocks in the projection pass
    ST = s // P          # s-tiles (context contraction / alpha columns)
    EJ = e // 512        # 512-wide output chunks of the context row

    nc = bacc.Bacc(
        "TRN2", target_bir_lowering=False, debug=False, num_devices=n_cores
    )
    encT_d = nc.dram_tensor("encT", [b_loc, e, s], f32r, kind="ExternalInput")
    encN_d = nc.dram_tensor("encN", [b_loc, s, e], f32r, kind="ExternalInput")
    wencT_d = nc.dram_tensor("wencT", [e, h], f32r, kind="ExternalInput")
    wdecT_d = nc.dram_tensor("wdecT", [h, h], f32r, kind="ExternalInput")
    # decm[p, dt*b_loc + b] = dec[b, dt*128 + p]; wem[p, ht] = W_energy[ht*128 + p]
    dec_d = nc.dram_tensor("decm", [P, (h // P) * b_loc], f32r, kind="ExternalInput")
    we_d = nc.dram_tensor("wem", [P, h // P], f32r, kind="ExternalInput")
    amask_d = nc.dram_tensor("amask", [b_loc, s], f32, kind="ExternalInput")
    ctx_d = nc.dram_tensor("ctx", [b_loc, e], f32, kind="ExternalOutput")
    alph_d = nc.dram_tensor("alph", [b_loc, s], f32, kind="ExternalOutput")

    with tile.TileContext(nc) as tc:
        with tc.tile_pool(name="persist", bufs=1) as persist:
            wenc_sb = persist.tile([P, ET, h], f32r)
            nc.sync.dma_start(
                out=wenc_sb[:],
                in_=wencT_d.rearrange("(et p) h -> p et h", p=P),
            )
            we_sb = persist.tile([P, HT], f32r)
            nc.sync.dma_start(out=we_sb[:], in_=we_d[:, :])
            one_sb = persist.tile([1, 1], f32)
            nc.vector.memset(one_sb, 1.0)
            decp_sb = persist.tile([P, HT * b_loc], f32)

            # dec projection: decp[h, b] = sum_d W_dec[h, d] * dec[b, d]
            with tc.tile_pool(name="decw", bufs=1) as decw, tc.tile_pool(
                name="decps", bufs=1, space="PSUM"
            ) as decps:
                wdec_sb = decw.tile([P, DT, h], f32r)
                nc.sync.dma_start(
                    out=wdec_sb[:],
                    in_=wdecT_d.rearrange("(dt p) h -> p dt h", p=P),
                )
                dec_sb = decw.tile([P, DT * b_loc], f32r)
                nc.sync.dma_start(out=dec_sb[:], in_=dec_d[:, :])
                psd = decps.tile([P, HT * b_loc], f32)
                for ht in range(HT):
                    for dt in range(DT):
                        nc.tensor.matmul(
                            psd[:, ht * b_loc : (ht + 1) * b_loc],
                            wdec_sb[:, dt, ht * P : (ht + 1) * P],
                            dec_sb[:, dt * b_loc : (dt + 1) * b_loc],
                            start=(dt == 0),
                            stop=(dt == DT - 1),
                        )
                nc.vector.tensor_copy(decp_sb[:], psd[:])

            with (
                tc.tile_pool(name="te", bufs=2) as te_pool,
                tc.tile_pool(name="tt", bufs=2) as t_pool,
                tc.tile_pool(name="en", bufs=2) as en_pool,
                tc.tile_pool(name="rows", bufs=2) as rows,
                tc.tile_pool(name="small", bufs=2) as small,
                tc.tile_pool(name="pp", bufs=2, space="PSUM") as pp,
                tc.tile_pool(name="pe", bufs=1, space="PSUM") as pe,
                tc.tile_pool(name="pa", bufs=1, space="PSUM") as pa,
                tc.tile_pool(name="pc", bufs=1, space="PSUM") as pc,
            ):
                for b in [bb for _ in range(n_iter) for bb in range(b_loc)]:
                    erow = rows.tile([1, s], f32, tag="erow", bufs=1)
                    amrow = rows.tile([1, s], f32, tag="amrow", bufs=1)
                    nc.sync.dma_start(out=amrow[:], in_=amask_d[b])
                    for sb in range(NSB):
                        te = te_pool.tile([P, ET, sblk], f32r, tag="te")
                        nc.sync.dma_start(
                            out=te[:],
                            in_=encT_d[b].rearrange("(et p) s -> p et s", p=P)[
                                :, :, sb * sblk : (sb + 1) * sblk
                            ],
                        )
                        pet = pe.tile([1, sblk], f32, tag="pet")
                        for ht in range(HT):
                            ppt = pp.tile([P, sblk], f32, tag="ppt")
                            for et in range(ET):
                                nc.tensor.matmul(
                                    ppt[:],
                                    wenc_sb[:, et, ht * P : (ht + 1) * P],
                                    te[:, et, :],
                                    start=(et == 0),
                                    stop=(et == ET - 1),
                                )
                            tt = t_pool.tile([P, sblk], f32r, tag="tt", bufs=4)
                            nc.scalar.activation(
                                tt[:],
                                ppt[:],
                                AF.Tanh,
                                bias=decp_sb[:, ht * b_loc + b : ht * b_loc + b + 1],
                            )
                            nc.tensor.matmul(
                                pet[:],
                                we_sb[:, ht : ht + 1],
                                tt[:],
                                start=(ht == 0),
                                stop=(ht == HT - 1),
                            )
                        nc.vector.tensor_add(
                            erow[0:1, sb * sblk : (sb + 1) * sblk],
                            pet[:],
                            amrow[0:1, sb * sblk : (sb + 1) * sblk],
                        )
                    # masked softmax on the [1, s] energies row
                    mx = small.tile([1, 1], f32, tag="mx")
                    nc.vector.reduce_max(mx[:], erow[:], axis=mybir.AxisListType.X)
                    nmx = small.tile([1, 1], f32, tag="nmx")
                    nc.vector.tensor_scalar_mul(nmx[:], mx[:], -1.0)
                    zs = small.tile([1, 1], f32, tag="zs")
                    arow = rows.tile([1, s], f32, tag="arow", bufs=1)
                    nc.scalar.activation(
                        arow[:], erow[:], AF.Exp, bias=nmx[0:1, 0:1], accum_out=zs[:]
                    )
                    rz = small.tile([1, 1], f32, tag="rz")
                    nc.vector.reciprocal(rz[:], zs[:])
                    anrow = rows.tile([1, s], f32, tag="anrow")
                    nc.vector.tensor_scalar_mul(anrow[:], arow[:], rz[0:1, 0:1])
                    nc.sync.dma_start(out=alph_d[b], in_=anrow[:])
                    # transpose alphas row into columns via PE transpose
                    pat = pa.tile([P, ST], f32, tag="pat")
                    for k in range(ST):
                        nc.tensor.matmul(
                            pat[:, k : k + 1],
                            anrow[0:1, k * P : (k + 1) * P],
                            one_sb[:],
                            is_transpose=True,
                            start=True,
                            stop=True,
                        )
                    acol = small.tile([P, ST], f32r, tag="acol")
                    nc.vector.tensor_copy(acol[:], pat[:])
                    # context pass over natural-layout enc blocks
                    pct = pc.tile([1, e], f32, tag="pct")
                    for k in range(ST):
                        en = en_pool.tile([P, e], f32r, tag="en")
                        nc.sync.dma_start(
                            out=en[:], in_=encN_d[b, k * P : (k + 1) * P, :]
                        )
                        for j in range(EJ):
                            nc.tensor.matmul(
                                pct[0:1, j * 512 : (j + 1) * 512],
                                acol[:, k : k + 1],
                                en[:, j * 512 : (j + 1) * 512],
                                start=(k == 0),
                                stop=(k == ST - 1),
                            )
                    crow = rows.tile([1, e], f32, tag="crow", bufs=1)
                    nc.scalar.activation(crow[:], pct[:], AF.Copy)
                    nc.sync.dma_start(out=ctx_d[b], in_=crow[:])
    nc.compile()
    return nc


_prog = None


def _get_prog():
    global _prog
    if _prog is None:
        _prog = build_program()
    return _prog


def _prepare_in_maps(inputs):
    return _build_in_maps(
        np.asarray(inputs["dec_state"], dtype=np.float32),
        np.asarray(inputs["enc_state"], dtype=np.float32),
        np.asarray(inputs["src_length"]),
        np.asarray(inputs["W_enc"], dtype=np.float32),
        np.asarray(inputs["W_dec"], dtype=np.float32),
        np.asarray(inputs["W_energy"], dtype=np.float32),
    )


def _build_in_maps(dec_state, enc_state, src_length, W_enc, W_dec, W_energy):
    wencT = np.ascontiguousarray(W_enc.T)
    wdecT = np.ascontiguousarray(W_dec.T)
    # wem[p, ht] = W_energy[0, ht*128 + p]
    wEm = np.ascontiguousarray(W_energy[0].reshape(H // P, P).T)
    iota = np.arange(S, dtype=np.int64)

    in_maps = []
    for c in range(NCORES):
        sl = slice(c * B_LOC, (c + 1) * B_LOC)
        encc = enc_state[sl]
        lens = src_length[sl].astype(np.int64)
        amask = np.where(iota[None, :] < lens[:, None], np.float32(0.0), NEG)
        # decm[p, dt*b_loc + b] = dec_state[c*B_LOC + b, 0, dt*128 + p]
        decm = np.ascontiguousarray(
            dec_state[sl, 0, :].reshape(B_LOC, H // P, P).transpose(2, 1, 0)
            .reshape(P, (H // P) * B_LOC)
        )
        in_maps.append(
            {
                "encT": np.ascontiguousarray(encc.transpose(0, 2, 1)),
                "encN": np.ascontiguousarray(encc),
                "wencT": wencT,
                "wdecT": wdecT,
                "decm": decm,
                "wem": wEm,
                "amask": amask.astype(np.float32),
            }
        )
    return in_maps


def kernel(dec_state, enc_state, src_length, W_enc, W_dec, W_energy):
    in_maps = _build_in_maps(
        np.asarray(dec_state, dtype=np.float32),
        np.asarray(enc_state, dtype=np.float32),
        np.asarray(src_length),
        np.asarray(W_enc, dtype=np.float32),
        np.asarray(W_dec, dtype=np.float32),
        np.asarray(W_energy, dtype=np.float32),
    )
    nc = _get_prog()
    try:
        res = run_bass_kernel_spmd(nc, in_maps, list(range(NCORES)))
    except Exception:
        res = run_bass_kernel_spmd(nc, in_maps, list(range(NCORES)))
    ctx = np.concatenate([r["ctx"] for r in res.results], 0).reshape(B, 1, E)
    alph = np.concatenate([r["alph"] for r in res.results], 0).reshape(B, 1, S)
    return ctx, alph

